# revision 1
# baseline (speedup 1.0000x reference)
"""2-layer GCN (PyG GCNConv x2 + ReLU) on 8 Trainium2 NeuronCores.

Strategy (dst-stationary, one program for all cores):
  - Pad nodes to NB blocks of 128; assign BC=NB/8 blocks per core (snake-dealt
    by in-degree so per-slot chunk quotas are tight across cores).
  - Per layer: dense transform G = H @ W (layer 1 replicated on every core,
    layer 2 only for own blocks + fp16 AllGather).
  - Sparse aggregation out[d] = sum_e norm_e * G[src_e] done per dst block as
    PE matmuls: S_chunk[e, d] = norm_e * (dstloc_e == d) built on DVE in one
    tensor_scalar (is_equal then mult), G_chunk rows fetched with dma_gather
    (batched int16 row gather).
  - dma_gather limits: idx < 32768 (int16) and source byte offset < 8MB, so
    every gather table is a separate tensor of <= 32768 fp16 rows read at
    offset 0: g1 is stored as g1a (first 32768 nodes) + g1b (rest); layer-2
    features are allgathered into g2full_a (first SA slots of each core) +
    g2full_b (remaining slots).
  - Layer-1 sparse runs "swapped" (psum[f,d] = G_chunk^T @ S) so the layer-2
    dense matmul consumes h^T directly - no transposes anywhere.
"""
import math
import numpy as np

import concourse.bass as bass
import concourse.bacc as bacc
import concourse.tile as tile
import concourse.mybir as mybir
from concourse.bass_utils import run_bass_kernel_spmd

P = 128
NCORE = 8
HALF_PROD = 32768
GS = 7  # slots per gather group


def _wrap_idx_call(flat):
    """flat [n] (n % 16 == 0) -> wrapped [128, n//16] int16; element i at [i%16, i//16]."""
    w = flat.reshape(-1, 16).T.astype(np.int16)
    return np.tile(w, (8, 1))


def _preprocess(x, edge_index, W1, b1, W2, b2, half=HALF_PROD):
    N, D = x.shape
    assert D == P
    NB = math.ceil(N / P)
    NB = math.ceil(NB / NCORE) * NCORE
    NPAD = NB * P
    BC = NB // NCORE
    # layer-2 table split: first SA slots of each core go to table A
    SA = min(half // (NCORE * P), BC)
    NA2 = NCORE * SA * P
    NB2 = NCORE * (BC - SA) * P

    src = np.concatenate([edge_index[0], np.arange(N)]).astype(np.int64)
    dst = np.concatenate([edge_index[1], np.arange(N)]).astype(np.int64)
    deg = np.bincount(dst, minlength=N).astype(np.float64)
    dinv = np.where(deg > 0, 1.0 / np.sqrt(deg), 0.0)
    norm = (dinv[src] * dinv[dst]).astype(np.float32)

    # --- block assignment: snake-deal blocks (sorted by edge count desc)
    eblk = dst // P
    blk_cnt = np.bincount(eblk, minlength=NB)
    order = np.argsort(-blk_cnt, kind="stable")
    blk = np.zeros((NCORE, BC), dtype=np.int64)
    for rank, b in enumerate(order):
        rnd, posn = divmod(rank, NCORE)
        c = posn if rnd % 2 == 0 else NCORE - 1 - posn
        blk[c, rnd] = b

    # node -> (core, slot)
    core_of_block = np.zeros(NB, dtype=np.int64)
    slot_of_block = np.zeros(NB, dtype=np.int64)
    for c in range(NCORE):
        for j in range(BC):
            core_of_block[blk[c, j]] = c
            slot_of_block[blk[c, j]] = j

    nodes = np.arange(NPAD)
    nb_of = nodes // P
    # layer 1: tables split by natural node id at `half`
    is_lo1 = nodes < half
    idxval1 = np.where(is_lo1, nodes, nodes - half)
    # layer 2: split by owner slot < SA; index within table
    cj, jj, pp = core_of_block[nb_of], slot_of_block[nb_of], nodes % P
    is_lo2 = jj < SA
    idxval2 = np.where(is_lo2, cj * SA * P + jj * P + pp,
                       cj * (BC - SA) * P + (jj - SA) * P + pp)

    _eorder = np.argsort(eblk, kind="stable")
    _bounds = np.searchsorted(eblk[_eorder], np.arange(NB + 1))
    edge_by_block = [_eorder[_bounds[b] : _bounds[b + 1]] for b in range(NB)]

    def build_layer(is_lo, idxval):
        ilo_v, ihi_v = idxval[src], idxval[src]
        m_all = is_lo[src]
        lo_idx = [[None] * BC for _ in range(NCORE)]
        hi_idx = [[None] * BC for _ in range(NCORE)]
        for c in range(NCORE):
            for j in range(BC):
                es = edge_by_block[blk[c, j]]
                dloc = (dst[es] % P).astype(np.float32)
                nrm = norm[es]
                m = m_all[es]
                for is_l, sel in ((True, m), (False, ~m)):
                    e_p = (ilo_v if is_l else ihi_v)[es][sel]
                    o = np.argsort(e_p, kind="stable")
                    pack = (e_p[o].astype(np.int16), dloc[sel][o], nrm[sel][o])
                    if is_l:
                        lo_idx[c][j] = pack
                    else:
                        hi_idx[c][j] = pack
        K_lo = [max(math.ceil(len(lo_idx[c][j][0]) / P) for c in range(NCORE)) for j in range(BC)]
        K_hi = [max(math.ceil(len(hi_idx[c][j][0]) / P) for c in range(NCORE)) for j in range(BC)]
        return lo_idx, hi_idx, K_lo, K_hi

    layers = [build_layer(is_lo1, idxval1), build_layer(is_lo2, idxval2)]
    groups = [list(range(s, min(s + GS, BC))) for s in range(0, BC, GS)]

    def pack_layer(lo_idx, hi_idx, K_lo, K_hi):
        NLO = sum(K_lo) * P
        NHI = sum(K_hi) * P
        TC = sum(K_lo) + sum(K_hi)
        idxlo = np.zeros((NCORE, 128, max(NLO // 16, 1)), dtype=np.int16)
        idxhi = np.zeros((NCORE, 128, max(NHI // 16, 1)), dtype=np.int16)
        dstloc = np.full((NCORE, 128, max(TC, 1)), -1.0, dtype=np.float32)
        nrm = np.zeros((NCORE, 128, max(TC, 1)), dtype=np.float32)
        for c in range(NCORE):
            col_lo = col_hi = tch = 0
            for g in groups:
                for j in g:
                    for is_l in (True, False):
                        e, d, nm = (lo_idx if is_l else hi_idx)[c][j]
                        K = (K_lo if is_l else K_hi)[j]
                        q = K * P
                        if q == 0:
                            continue
                        buf = np.zeros(q, dtype=np.int16)
                        buf[: len(e)] = e
                        wr = _wrap_idx_call(buf)
                        if is_l:
                            idxlo[c, :, col_lo * 8 : col_lo * 8 + q // 16] = wr
                            col_lo += K
                        else:
                            idxhi[c, :, col_hi * 8 : col_hi * 8 + q // 16] = wr
                            col_hi += K
                        dcol = np.full(q, -1.0, np.float32)
                        ncol = np.zeros(q, np.float32)
                        dcol[: len(d)] = d
                        ncol[: len(nm)] = nm
                        dstloc[c, :, tch : tch + K] = dcol.reshape(-1, P).T
                        nrm[c, :, tch : tch + K] = ncol.reshape(-1, P).T
                        tch += K
        return dict(idxlo=idxlo, idxhi=idxhi, dstloc=dstloc, nrm=nrm,
                    K_lo=K_lo, K_hi=K_hi, NLO=NLO, NHI=NHI, TC=TC)

    pk = [pack_layer(*layers[0]), pack_layer(*layers[1])]

    xpad = np.zeros((NPAD, P), dtype=np.float32)
    xpad[:N] = x
    xT16 = np.ascontiguousarray(xpad.T).astype(np.float16)

    cfg = dict(
        N=N, NB=NB, NPAD=NPAD, BC=BC, half=half, groups=groups, SA=SA,
        NA2=NA2, NB2=NB2,
        K1_lo=pk[0]["K_lo"], K1_hi=pk[0]["K_hi"],
        K2_lo=pk[1]["K_lo"], K2_hi=pk[1]["K_hi"],
        NLO1=pk[0]["NLO"], NHI1=pk[0]["NHI"], TC1=pk[0]["TC"],
        NLO2=pk[1]["NLO"], NHI2=pk[1]["NHI"], TC2=pk[1]["TC"],
    )

    iota = np.broadcast_to(np.arange(P, dtype=np.float32), (P, P)).copy()
    in_maps = []
    for c in range(NCORE):
        in_maps.append(dict(
            xT=xT16,
            W1=W1.astype(np.float16), W2=W2.astype(np.float16),
            b1c=b1.astype(np.float32).reshape(P, 1),
            b2r=b2.astype(np.float16).reshape(1, P),
            ones=np.ones((1, P), dtype=np.float16),
            iota=iota,
            idxlo1=pk[0]["idxlo"][c], idxhi1=pk[0]["idxhi"][c],
            idxlo2=pk[1]["idxlo"][c], idxhi2=pk[1]["idxhi"][c],
            dl1=pk[0]["dstloc"][c], nm1=pk[0]["nrm"][c],
            dl2=pk[1]["dstloc"][c], nm2=pk[1]["nrm"][c],
        ))
    return cfg, in_maps, blk


# --------------------------------------------------------------- device side

def build_program(cfg, stop_after="full", reps=1):
    NB, NPAD, BC, half, SA = cfg["NB"], cfg["NPAD"], cfg["BC"], cfg["half"], cfg["SA"]
    NA2, NB2 = cfg["NA2"], cfg["NB2"]
    groups = cfg["groups"]
    K1_lo, K1_hi = cfg["K1_lo"], cfg["K1_hi"]
    K2_lo, K2_hi = cfg["K2_lo"], cfg["K2_hi"]
    f16, f32, i16 = mybir.dt.float16, mybir.dt.float32, mybir.dt.int16

    nc = bacc.Bacc(num_devices=NCORE, num_swdge_queues=4)
    xT = nc.dram_tensor("xT", [P, NPAD], f16, kind="ExternalInput")
    W1 = nc.dram_tensor("W1", [P, P], f16, kind="ExternalInput")
    W2 = nc.dram_tensor("W2", [P, P], f16, kind="ExternalInput")
    b1c = nc.dram_tensor("b1c", [P, 1], f32, kind="ExternalInput")
    b2r = nc.dram_tensor("b2r", [1, P], f16, kind="ExternalInput")
    ones = nc.dram_tensor("ones", [1, P], f16, kind="ExternalInput")
    iota = nc.dram_tensor("iota", [P, P], f32, kind="ExternalInput")
    idxlo1 = nc.dram_tensor("idxlo1", [128, max(cfg["NLO1"] // 16, 1)], i16, kind="ExternalInput")
    idxhi1 = nc.dram_tensor("idxhi1", [128, max(cfg["NHI1"] // 16, 1)], i16, kind="ExternalInput")
    idxlo2 = nc.dram_tensor("idxlo2", [128, max(cfg["NLO2"] // 16, 1)], i16, kind="ExternalInput")
    idxhi2 = nc.dram_tensor("idxhi2", [128, max(cfg["NHI2"] // 16, 1)], i16, kind="ExternalInput")
    dl1 = nc.dram_tensor("dl1", [128, max(cfg["TC1"], 1)], f32, kind="ExternalInput")
    nm1 = nc.dram_tensor("nm1", [128, max(cfg["TC1"], 1)], f32, kind="ExternalInput")
    dl2 = nc.dram_tensor("dl2", [128, max(cfg["TC2"], 1)], f32, kind="ExternalInput")
    nm2 = nc.dram_tensor("nm2", [128, max(cfg["TC2"], 1)], f32, kind="ExternalInput")

    y = nc.dram_tensor("y", [BC * P, P], f32, kind="ExternalOutput")

    g1a = nc.dram_tensor("g1a", [half, P], f16)
    g1b = nc.dram_tensor("g1b", [NPAD - half, P], f16)
    g2own = nc.dram_tensor("g2own", [BC * P, P], f16)
    g2fa = nc.dram_tensor("g2fa", [max(NA2, P), P], f16, addr_space="Shared")
    g2fb = nc.dram_tensor("g2fb", [max(NB2, P), P], f16, addr_space="Shared")

    with tile.TileContext(nc) as tc:
        with (
            tc.tile_pool(name="const", bufs=1) as cpool,
            tc.tile_pool(name="meta", bufs=1) as mpool,
            tc.tile_pool(name="xin", bufs=3) as xpool,
            tc.tile_pool(name="g1out", bufs=3) as g1pool,
            tc.tile_pool(name="idx", bufs=3) as ipool,
            tc.tile_pool(name="gath", bufs=3) as gpool,
            tc.tile_pool(name="s", bufs=6) as spool,
            tc.tile_pool(name="h", bufs=3) as hpool,
            tc.tile_pool(name="oeps", bufs=3) as opool,
            tc.tile_pool(name="ps_sp", bufs=3, space="PSUM") as pssp,
            tc.tile_pool(name="ps_dn", bufs=2, space="PSUM") as psdn,
        ):
            w1t = cpool.tile([P, P], f16)
            nc.sync.dma_start(w1t[:], W1[:])
            w2t = cpool.tile([P, P], f16)
            nc.sync.dma_start(w2t[:], W2[:])
            b1t = cpool.tile([P, 1], f32)
            nc.sync.dma_start(b1t[:], b1c[:])
            b2t = cpool.tile([1, P], f16)
            nc.sync.dma_start(b2t[:], b2r[:])
            onest = cpool.tile([1, P], f16)
            nc.sync.dma_start(onest[:], ones[:])
            iot = cpool.tile([P, P], f32)
            nc.sync.dma_start(iot[:], iota[:])

            def _one_pass():
                # ---- phase 1: g1 = x @ W1 for ALL blocks (replicated per core)
                skip_p1 = stop_after.startswith("g-")
                xb = 28 if NB % 28 == 0 else 8
                assert NB % xb == 0
                for t in range(0 if skip_p1 else NB // xb, NB // xb) if False else range(NB // xb):
                    if skip_p1:
                        break
                    xt = xpool.tile([P, xb * P], f16, tag="xt")
                    nc.sync.dma_start(xt[:], xT[:, t * xb * P : (t + 1) * xb * P])
                    g1sb = g1pool.tile([P, xb, P], f16, tag="g1sb")
                    for k in range(xb):
                        ps = psdn.tile([P, P], f32, space="PSUM", tag="psd")
                        nc.tensor.matmul(
                            out=ps[:], lhsT=xt[:, k * P : (k + 1) * P], rhs=w1t[:],
                            start=True, stop=True,
                        )
                        nc.vector.tensor_copy(g1sb[:, k, :], ps[:])
                    r0, r1 = t * xb * P, (t + 1) * xb * P
                    if r1 <= half:
                        nc.sync.dma_start(
                            g1a[r0:r1, :].rearrange("(k p) f -> p k f", p=P), g1sb[:])
                    elif r0 >= half:
                        nc.sync.dma_start(
                            g1b[r0 - half : r1 - half, :].rearrange("(k p) f -> p k f", p=P),
                            g1sb[:])
                    else:
                        ka = (half - r0) // P
                        nc.sync.dma_start(
                            g1a[r0:half, :].rearrange("(k p) f -> p k f", p=P),
                            g1sb[:, :ka, :])
                        nc.sync.dma_start(
                            g1b[: r1 - half, :].rearrange("(k p) f -> p k f", p=P),
                            g1sb[:, ka:, :])

                # ---- sparse layer helper
                def sparse_layer(src_lo, src_hi, K_lo, K_hi, idxlo_d, idxhi_d, dl_d, nm_d,
                                 layer, sub="full", after_group=None):
                    TC = max(sum(K_lo) + sum(K_hi), 1)
                    dlt = mpool.tile([P, TC], f32, tag=f"dl{layer}", name=f"dl{layer}")
                    nc.sync.dma_start(dlt[:], dl_d[:])
                    nmt = mpool.tile([P, TC], f32, tag=f"nm{layer}", name=f"nm{layer}")
                    nc.sync.dma_start(nmt[:], nm_d[:])
                    tch = 0
                    col_lo = col_hi = 0
                    for g in groups:
                        QLO = sum(K_lo[j] for j in g)
                        QHI = sum(K_hi[j] for j in g)
                        lob = gpool.tile([P, max(QLO, 1), P], f16, tag="lob", name="lob")
                        hib = gpool.tile([P, max(QHI, 1), P], f16, tag="hib", name="hib")
                        if QLO:
                            ilo = ipool.tile([P, QLO * 8], i16, tag="ilo", name="ilo")
                            nc.sync.dma_start(ilo[:], idxlo_d[:, col_lo * 8 : (col_lo + QLO) * 8])
                            nc.gpsimd.dma_gather(
                                out_ap=lob[:], in_ap=src_lo[:, :], idxs_ap=ilo[:],
                                num_idxs=QLO * P, num_idxs_reg=QLO * P, elem_size=P,
                                single_packet=False,
                            )
                        if QHI:
                            ihi = ipool.tile([P, QHI * 8], i16, tag="ihi", name="ihi")
                            nc.sync.dma_start(ihi[:], idxhi_d[:, col_hi * 8 : (col_hi + QHI) * 8])
                            nc.gpsimd.dma_gather(
                                out_ap=hib[:], in_ap=src_hi[:, :], idxs_ap=ihi[:],
                                num_idxs=QHI * P, num_idxs_reg=QHI * P, elem_size=P,
                                single_packet=False,
                            )
                        if sub == "gather":
                            col_lo += QLO
                            col_hi += QHI
                            continue
                        if layer == 2:
                            out_sb = opool.tile([P, len(g), P], f32, tag="osb", name="osb")
                        else:
                            out_sb = opool.tile([P, len(g), P], f16, tag="g2sb", name="g2sb")
                        glo = ghi = 0
                        for ji, j in enumerate(g):
                            nch = K_lo[j] + K_hi[j] + (1 if layer == 2 else 0)
                            ps1 = pssp.tile([P, P], f32, space="PSUM", tag="ps_sp", name="ps_sp")
                            ci = 0
                            for half_sel in (0, 1):
                                cnt = K_lo[j] if half_sel == 0 else K_hi[j]
                                buf = lob if half_sel == 0 else hib
                                base = glo if half_sel == 0 else ghi
                                for q in range(cnt):
                                    st = spool.tile([P, P], f16, tag="S", name="S")
                                    nc.vector.tensor_scalar(
                                        out=st[:], in0=iot[:],
                                        scalar1=dlt[:, tch : tch + 1],
                                        scalar2=nmt[:, tch : tch + 1],
                                        op0=mybir.AluOpType.is_equal,
                                        op1=mybir.AluOpType.mult,
                                    )
                                    chunk = buf[:, base + q, :]
                                    if layer == 1:
                                        nc.tensor.matmul(
                                            out=ps1[:], lhsT=chunk, rhs=st[:],
                                            start=(ci == 0), stop=(ci == nch - 1),
                                        )
                                    else:
                                        nc.tensor.matmul(
                                            out=ps1[:], lhsT=st[:], rhs=chunk,
                                            start=(ci == 0), stop=(ci == nch - 1),
                                        )
                                    ci += 1
                                    tch += 1
                            glo += K_lo[j]
                            ghi += K_hi[j]
                            if layer == 1:
                                hT = hpool.tile([P, P], f16, tag="hT", name="hT")
                                nc.scalar.activation(
                                    hT[:], ps1[:], mybir.ActivationFunctionType.Relu,
                                    bias=b1t[:, :1],
                                )
                                ps2 = psdn.tile([P, P], f32, space="PSUM", tag="psd", name="psd")
                                nc.tensor.matmul(out=ps2[:], lhsT=hT[:], rhs=w2t[:],
                                                 start=True, stop=True)
                                nc.vector.tensor_copy(out_sb[:, ji, :], ps2[:])
                            else:
                                nc.tensor.matmul(out=ps1[:], lhsT=onest[:], rhs=b2t[:],
                                                 start=False, stop=True)
                                nc.vector.tensor_copy(out_sb[:, ji, :], ps1[:])
                        g0 = g[0]
                        dst_d = g2own if layer == 1 else y
                        nc.sync.dma_start(
                            dst_d[g0 * P : (g0 + len(g)) * P, :].rearrange("(k p) f -> p k f", p=P),
                            out_sb[:],
                        )
                        col_lo += QLO
                        col_hi += QHI
                        if after_group is not None:
                            after_group(groups.index(g))

                # ---- phase 2: layer-1 sparse + layer-2 dense (own blocks)
                if stop_after != "p1":
                    if stop_after.startswith("g-"):
                        sub = "gather"
                    else:
                        sub = stop_after[3:] if stop_after.startswith("p2-") else "full"
                    # group index whose output completes g2own[:SA*P] (AG_a input)
                    ag_a_group = next(gi for gi, g in enumerate(groups) if SA - 1 in g) \
                        if SA - 1 < BC else len(groups) - 1

                    def _after_group(gi):
                        if gi == ag_a_group:
                            nc.gpsimd.collective_compute(
                                "AllGather", mybir.AluOpType.bypass,
                                replica_groups=[list(range(NCORE))],
                                ins=[g2own[: SA * P, :]], outs=[g2fa[:NA2, :]],
                            )
                    gather_times = 2 if stop_after == "g-2x" else 1
                    for _gt in range(gather_times):
                        sparse_layer(g1a, g1b, K1_lo, K1_hi, idxlo1, idxhi1, dl1, nm1,
                                     layer=1, sub=sub,
                                     after_group=_after_group if stop_after == "full" else None)

                # ---- phase 3: allgather g2 (two tables, offset-0 gather sources)
                if stop_after in ("full", "noag"):
                  if stop_after == "full":
                    if NB2 > 0:
                        nc.gpsimd.collective_compute(
                            "AllGather", mybir.AluOpType.bypass,
                            replica_groups=[list(range(NCORE))],
                            ins=[g2own[SA * P :, :]], outs=[g2fb[:NB2, :]],
                        )
                  # ---- phase 4: layer-2 sparse -> y
                  sparse_layer(g2fa, g2fb, K2_lo, K2_hi, idxlo2, idxhi2, dl2, nm2,
                               layer=2)


            for _rep in range(reps):
                _one_pass()

    nc.compile()
    return nc


# ------------------------------------------------------------------- driver

def _run(x, edge_index, W1, b1, W2, b2, half=HALF_PROD, trace=False, stop_after="full"):
    cfg, in_maps, blk = _preprocess(x, edge_index, W1, b1, W2, b2, half=half)
    nc = build_program(cfg, stop_after=stop_after)
    res = run_bass_kernel_spmd(nc, in_maps, list(range(NCORE)), trace=trace)
    N, BC = cfg["N"], cfg["BC"]
    Y = np.zeros((cfg["NPAD"], P), dtype=np.float32)
    for c in range(NCORE):
        yc = res.results[c]["y"]
        for j in range(BC):
            b = blk[c, j]
            Y[b * P : (b + 1) * P] = yc[j * P : (j + 1) * P]
    return Y[:N], res


def kernel(**inputs):
    x = np.asarray(inputs["x"], dtype=np.float32)
    edge_index = np.asarray(inputs["edge_index"])
    W1 = np.asarray(inputs["W1"], dtype=np.float32)
    b1 = np.asarray(inputs["b1"], dtype=np.float32)
    W2 = np.asarray(inputs["W2"], dtype=np.float32)
    b2 = np.asarray(inputs["b2"], dtype=np.float32)
    Y, _ = _run(x, edge_index, W1, b1, W2, b2)
    return Y



# revision 3
# speedup vs baseline: 2.6185x; 2.6185x over previous
"""2-layer GCN (PyG GCNConv x2 + ReLU) on 8 Trainium2 NeuronCores.

Strategy (dst-stationary, one program for all cores):
  - Pad nodes to NB blocks of 128; assign BC=NB/8 blocks per core (snake-dealt
    by in-degree so per-slot chunk quotas are tight across cores).
  - Layer 1 aggregates x FIRST (aggregation is linear), then applies W1 ->
    relu -> W2 per dst block: no replicated dense phase, gathers read a
    host-prepared fp16 x table directly.
  - Sparse aggregation per dst block as PE matmuls: S_chunk[e, d] =
    norm_e * (dstloc_e == d) built on DVE in one tensor_scalar (fp16), chunk
    rows fetched with dma_gather (int16 row gather), SPLIT into many small
    gather instructions round-robined across all 4 SWDGE queues (a single
    queue serializes at ~64 GB/s; 4 queues x small pieces reach ~460 GB/s).
  - dma_gather limits: idx < 32768 (int16) and source byte offset < 8MB, so
    every gather table is <= 32768 fp16 rows read at offset 0: x is stored as
    xa (first 32768 nodes) + xb (rest); layer-2 features are allgathered into
    g2fa (first SA slots of each core) + g2fb (remaining slots).
  - Layer-1 sparse runs "swapped" (psum[f, d] = chunk^T @ S) so W1 can be
    applied on the aggregate via psum2 = W1^T @ aggT, relu+b1 -> hT, then
    psum3 = hT^T @ W2 = (h @ W2) per block - no transposes anywhere.
"""
import math
import numpy as np

import concourse.bass as bass
import concourse.bacc as bacc
import concourse.tile as tile
import concourse.mybir as mybir
from concourse.bass_utils import run_bass_kernel_spmd

P = 128
NCORE = 8
HALF_PROD = 32768
GS = 7       # slots per gather group
GRAIN = 13   # chunks per gather instruction (~1664 idxs)
NQ = 4       # SWDGE queues


def _wrap_idx_call(flat):
    """flat [n] (n % 16 == 0) -> wrapped [128, n//16] int16; element i at [i%16, i//16]."""
    w = flat.reshape(-1, 16).T.astype(np.int16)
    return np.tile(w, (8, 1))


def _preprocess(x, edge_index, W1, b1, W2, b2, half=HALF_PROD):
    N, D = x.shape
    assert D == P
    NB = math.ceil(N / P)
    NB = math.ceil(NB / NCORE) * NCORE
    NPAD = NB * P
    BC = NB // NCORE
    # layer-2 table split: first SA slots of each core go to table A
    SA = min(half // (NCORE * P), BC)
    NA2 = NCORE * SA * P
    NB2 = NCORE * (BC - SA) * P

    src = np.concatenate([edge_index[0], np.arange(N)]).astype(np.int64)
    dst = np.concatenate([edge_index[1], np.arange(N)]).astype(np.int64)
    deg = np.bincount(dst, minlength=N).astype(np.float64)
    dinv = np.where(deg > 0, 1.0 / np.sqrt(deg), 0.0)
    norm = (dinv[src] * dinv[dst]).astype(np.float32)

    # --- block assignment: snake-deal blocks (sorted by edge count desc)
    eblk = dst // P
    blk_cnt = np.bincount(eblk, minlength=NB)
    order = np.argsort(-blk_cnt, kind="stable")
    blk = np.zeros((NCORE, BC), dtype=np.int64)
    for rank, b in enumerate(order):
        rnd, posn = divmod(rank, NCORE)
        c = posn if rnd % 2 == 0 else NCORE - 1 - posn
        blk[c, rnd] = b

    # node -> (core, slot)
    core_of_block = np.zeros(NB, dtype=np.int64)
    slot_of_block = np.zeros(NB, dtype=np.int64)
    for c in range(NCORE):
        for j in range(BC):
            core_of_block[blk[c, j]] = c
            slot_of_block[blk[c, j]] = j

    nodes = np.arange(NPAD)
    nb_of = nodes // P
    # layer 1: tables split by natural node id at `half`
    is_lo1 = nodes < half
    idxval1 = np.where(is_lo1, nodes, nodes - half)
    # layer 2: split by owner slot < SA; index within table
    cj, jj, pp = core_of_block[nb_of], slot_of_block[nb_of], nodes % P
    is_lo2 = jj < SA
    idxval2 = np.where(is_lo2, cj * SA * P + jj * P + pp,
                       cj * (BC - SA) * P + (jj - SA) * P + pp)

    _eorder = np.argsort(eblk, kind="stable")
    _bounds = np.searchsorted(eblk[_eorder], np.arange(NB + 1))
    edge_by_block = [_eorder[_bounds[b] : _bounds[b + 1]] for b in range(NB)]

    def build_layer(is_lo, idxval):
        ilo_v, ihi_v = idxval[src], idxval[src]
        m_all = is_lo[src]
        lo_idx = [[None] * BC for _ in range(NCORE)]
        hi_idx = [[None] * BC for _ in range(NCORE)]
        for c in range(NCORE):
            for j in range(BC):
                es = edge_by_block[blk[c, j]]
                dloc = (dst[es] % P).astype(np.float32)
                nrm = norm[es]
                m = m_all[es]
                for is_l, sel in ((True, m), (False, ~m)):
                    e_p = (ilo_v if is_l else ihi_v)[es][sel]
                    o = np.argsort(e_p, kind="stable")
                    pack = (e_p[o].astype(np.int16), dloc[sel][o], nrm[sel][o])
                    if is_l:
                        lo_idx[c][j] = pack
                    else:
                        hi_idx[c][j] = pack
        K_lo = [max(math.ceil(len(lo_idx[c][j][0]) / P) for c in range(NCORE)) for j in range(BC)]
        K_hi = [max(math.ceil(len(hi_idx[c][j][0]) / P) for c in range(NCORE)) for j in range(BC)]
        return lo_idx, hi_idx, K_lo, K_hi

    layers = [build_layer(is_lo1, idxval1), build_layer(is_lo2, idxval2)]
    groups = [list(range(s, min(s + GS, BC))) for s in range(0, BC, GS)]

    def pack_layer(lo_idx, hi_idx, K_lo, K_hi):
        NLO = sum(K_lo) * P
        NHI = sum(K_hi) * P
        TC = sum(K_lo) + sum(K_hi)
        idxlo = np.zeros((NCORE, 128, max(NLO // 16, 1)), dtype=np.int16)
        idxhi = np.zeros((NCORE, 128, max(NHI // 16, 1)), dtype=np.int16)
        dstloc = np.full((NCORE, 128, max(TC, 1)), -1.0, dtype=np.float32)
        nrm = np.zeros((NCORE, 128, max(TC, 1)), dtype=np.float32)
        for c in range(NCORE):
            col_lo = col_hi = tch = 0
            for g in groups:
                for j in g:
                    for is_l in (True, False):
                        e, d, nm = (lo_idx if is_l else hi_idx)[c][j]
                        K = (K_lo if is_l else K_hi)[j]
                        q = K * P
                        if q == 0:
                            continue
                        buf = np.zeros(q, dtype=np.int16)
                        buf[: len(e)] = e
                        wr = _wrap_idx_call(buf)
                        if is_l:
                            idxlo[c, :, col_lo * 8 : col_lo * 8 + q // 16] = wr
                            col_lo += K
                        else:
                            idxhi[c, :, col_hi * 8 : col_hi * 8 + q // 16] = wr
                            col_hi += K
                        dcol = np.full(q, -1.0, np.float32)
                        ncol = np.zeros(q, np.float32)
                        dcol[: len(d)] = d
                        ncol[: len(nm)] = nm
                        dstloc[c, :, tch : tch + K] = dcol.reshape(-1, P).T
                        nrm[c, :, tch : tch + K] = ncol.reshape(-1, P).T
                        tch += K
        return dict(idxlo=idxlo, idxhi=idxhi, dstloc=dstloc, nrm=nrm,
                    K_lo=K_lo, K_hi=K_hi, NLO=NLO, NHI=NHI, TC=TC)

    pk = [pack_layer(*layers[0]), pack_layer(*layers[1])]

    xpad = np.zeros((NPAD, P), dtype=np.float16)
    xpad[:N] = x.astype(np.float16)

    cfg = dict(
        N=N, NB=NB, NPAD=NPAD, BC=BC, half=half, groups=groups, SA=SA,
        NA2=NA2, NB2=NB2,
        K1_lo=pk[0]["K_lo"], K1_hi=pk[0]["K_hi"],
        K2_lo=pk[1]["K_lo"], K2_hi=pk[1]["K_hi"],
        NLO1=pk[0]["NLO"], NHI1=pk[0]["NHI"], TC1=pk[0]["TC"],
        NLO2=pk[1]["NLO"], NHI2=pk[1]["NHI"], TC2=pk[1]["TC"],
    )

    iota = np.broadcast_to(np.arange(P, dtype=np.float16), (P, P)).copy()
    in_maps = []
    for c in range(NCORE):
        in_maps.append(dict(
            xa=xpad[:half], xb=xpad[half:],
            W1=W1.astype(np.float16), W2=W2.astype(np.float16),
            b1c=b1.astype(np.float32).reshape(P, 1),
            b2r=b2.astype(np.float16).reshape(1, P),
            ones=np.ones((1, P), dtype=np.float16),
            iota=iota,
            idxlo1=pk[0]["idxlo"][c], idxhi1=pk[0]["idxhi"][c],
            idxlo2=pk[1]["idxlo"][c], idxhi2=pk[1]["idxhi"][c],
            dl1=pk[0]["dstloc"][c], nm1=pk[0]["nrm"][c],
            dl2=pk[1]["dstloc"][c], nm2=pk[1]["nrm"][c],
        ))
    return cfg, in_maps, blk


# --------------------------------------------------------------- device side

def build_program(cfg, stop_after="full", reps=1):
    NB, NPAD, BC, half, SA = cfg["NB"], cfg["NPAD"], cfg["BC"], cfg["half"], cfg["SA"]
    NA2, NB2 = cfg["NA2"], cfg["NB2"]
    groups = cfg["groups"]
    K1_lo, K1_hi = cfg["K1_lo"], cfg["K1_hi"]
    K2_lo, K2_hi = cfg["K2_lo"], cfg["K2_hi"]
    f16, f32, i16 = mybir.dt.float16, mybir.dt.float32, mybir.dt.int16

    nc = bacc.Bacc(num_devices=NCORE, num_swdge_queues=NQ)
    xa = nc.dram_tensor("xa", [half, P], f16, kind="ExternalInput")
    xb = nc.dram_tensor("xb", [NPAD - half, P], f16, kind="ExternalInput")
    W1 = nc.dram_tensor("W1", [P, P], f16, kind="ExternalInput")
    W2 = nc.dram_tensor("W2", [P, P], f16, kind="ExternalInput")
    b1c = nc.dram_tensor("b1c", [P, 1], f32, kind="ExternalInput")
    b2r = nc.dram_tensor("b2r", [1, P], f16, kind="ExternalInput")
    ones = nc.dram_tensor("ones", [1, P], f16, kind="ExternalInput")
    iota = nc.dram_tensor("iota", [P, P], f16, kind="ExternalInput")
    idxlo1 = nc.dram_tensor("idxlo1", [128, max(cfg["NLO1"] // 16, 1)], i16, kind="ExternalInput")
    idxhi1 = nc.dram_tensor("idxhi1", [128, max(cfg["NHI1"] // 16, 1)], i16, kind="ExternalInput")
    idxlo2 = nc.dram_tensor("idxlo2", [128, max(cfg["NLO2"] // 16, 1)], i16, kind="ExternalInput")
    idxhi2 = nc.dram_tensor("idxhi2", [128, max(cfg["NHI2"] // 16, 1)], i16, kind="ExternalInput")
    dl1 = nc.dram_tensor("dl1", [128, max(cfg["TC1"], 1)], f32, kind="ExternalInput")
    nm1 = nc.dram_tensor("nm1", [128, max(cfg["TC1"], 1)], f32, kind="ExternalInput")
    dl2 = nc.dram_tensor("dl2", [128, max(cfg["TC2"], 1)], f32, kind="ExternalInput")
    nm2 = nc.dram_tensor("nm2", [128, max(cfg["TC2"], 1)], f32, kind="ExternalInput")

    y = nc.dram_tensor("y", [BC * P, P], f32, kind="ExternalOutput")

    g2own = nc.dram_tensor("g2own", [BC * P, P], f16)
    g2fa = nc.dram_tensor("g2fa", [max(NA2, P), P], f16, addr_space="Shared")
    g2fb = nc.dram_tensor("g2fb", [max(NB2, P), P], f16, addr_space="Shared")

    qrr = [0]  # round-robin SWDGE queue cursor

    with tile.TileContext(nc) as tc:
        with (
            tc.tile_pool(name="const", bufs=1) as cpool,
            tc.tile_pool(name="meta", bufs=1) as mpool,
            tc.tile_pool(name="idx", bufs=8) as ipool,
            tc.tile_pool(name="gath", bufs=3) as gpool,
            tc.tile_pool(name="s", bufs=6) as spool,
            tc.tile_pool(name="agg", bufs=3) as apool,
            tc.tile_pool(name="h", bufs=3) as hpool,
            tc.tile_pool(name="oeps", bufs=3) as opool,
            tc.tile_pool(name="ps_sp", bufs=4, space="PSUM") as pssp,
            tc.tile_pool(name="ps_dn", bufs=3, space="PSUM") as psdn,
        ):
            w1t = cpool.tile([P, P], f16)
            nc.sync.dma_start(w1t[:], W1[:])
            w2t = cpool.tile([P, P], f16)
            nc.sync.dma_start(w2t[:], W2[:])
            b1t = cpool.tile([P, 1], f32)
            nc.sync.dma_start(b1t[:], b1c[:])
            b2t = cpool.tile([1, P], f16)
            nc.sync.dma_start(b2t[:], b2r[:])
            onest = cpool.tile([1, P], f16)
            nc.sync.dma_start(onest[:], ones[:])
            iot = cpool.tile([P, P], f16)
            nc.sync.dma_start(iot[:], iota[:])

            def split_gather(buf, src_d, idx_d, col, Q, tag):
                """Gather Q chunks into buf[:, 0:Q, :], split into GRAIN-chunk
                pieces round-robined over the SWDGE queues."""
                off = 0
                while off < Q:
                    nqc = min(GRAIN, Q - off)
                    it = ipool.tile([P, nqc * 8], i16, tag=f"i{tag}", name=f"i{tag}")
                    nc.sync.dma_start(
                        it[:], idx_d[:, (col + off) * 8 : (col + off + nqc) * 8])
                    nc.gpsimd.dma_gather(
                        out_ap=buf[:, off : off + nqc, :], in_ap=src_d[:, :],
                        idxs_ap=it[:], num_idxs=nqc * P, num_idxs_reg=nqc * P,
                        elem_size=P, single_packet=False,
                        queue_num=qrr[0] % NQ,
                    )
                    qrr[0] += 1
                    off += nqc

            def sparse_layer(src_lo, src_hi, K_lo, K_hi, idxlo_d, idxhi_d, dl_d, nm_d,
                             layer, sub="full", after_group=None):
                TC = max(sum(K_lo) + sum(K_hi), 1)
                dlt = mpool.tile([P, TC], f32, tag=f"dl{layer}", name=f"dl{layer}")
                nc.sync.dma_start(dlt[:], dl_d[:])
                nmt = mpool.tile([P, TC], f32, tag=f"nm{layer}", name=f"nm{layer}")
                nc.sync.dma_start(nmt[:], nm_d[:])
                tch = 0
                col_lo = col_hi = 0
                for g in groups:
                    QLO = sum(K_lo[j] for j in g)
                    QHI = sum(K_hi[j] for j in g)
                    lob = gpool.tile([P, max(QLO, 1), P], f16, tag="lob", name="lob")
                    hib = gpool.tile([P, max(QHI, 1), P], f16, tag="hib", name="hib")
                    if QLO:
                        split_gather(lob, src_lo, idxlo_d, col_lo, QLO, "lo")
                    if QHI:
                        split_gather(hib, src_hi, idxhi_d, col_hi, QHI, "hi")
                    if sub == "gather":
                        col_lo += QLO
                        col_hi += QHI
                        continue
                    if layer == 2:
                        out_sb = opool.tile([P, len(g), P], f32, tag="osb", name="osb")
                    else:
                        out_sb = opool.tile([P, len(g), P], f16, tag="g2sb", name="g2sb")
                    glo = ghi = 0
                    for ji, j in enumerate(g):
                        nch = K_lo[j] + K_hi[j] + (1 if layer == 2 else 0)
                        ps1 = pssp.tile([P, P], f32, space="PSUM", tag="ps_sp", name="ps_sp")
                        ci = 0
                        for half_sel in (0, 1):
                            cnt = K_lo[j] if half_sel == 0 else K_hi[j]
                            buf = lob if half_sel == 0 else hib
                            base = glo if half_sel == 0 else ghi
                            for q in range(cnt):
                                st = spool.tile([P, P], f16, tag="S", name="S")
                                nc.vector.tensor_scalar(
                                    out=st[:], in0=iot[:],
                                    scalar1=dlt[:, tch : tch + 1],
                                    scalar2=nmt[:, tch : tch + 1],
                                    op0=mybir.AluOpType.is_equal,
                                    op1=mybir.AluOpType.mult,
                                )
                                chunk = buf[:, base + q, :]
                                if layer == 1:
                                    nc.tensor.matmul(
                                        out=ps1[:], lhsT=chunk, rhs=st[:],
                                        start=(ci == 0), stop=(ci == nch - 1),
                                    )
                                else:
                                    nc.tensor.matmul(
                                        out=ps1[:], lhsT=st[:], rhs=chunk,
                                        start=(ci == 0), stop=(ci == nch - 1),
                                    )
                                ci += 1
                                tch += 1
                        glo += K_lo[j]
                        ghi += K_hi[j]
                        if layer == 1:
                            # ps1 = aggT [f, d]; z1T = W1^T @ agg^T -> relu -> hT
                            aggs = apool.tile([P, P], f16, tag="aggs", name="aggs")
                            nc.vector.tensor_copy(aggs[:], ps1[:])
                            ps2 = psdn.tile([P, P], f32, space="PSUM", tag="psd", name="psd")
                            nc.tensor.matmul(out=ps2[:], lhsT=w1t[:], rhs=aggs[:],
                                             start=True, stop=True)
                            hT = hpool.tile([P, P], f16, tag="hT", name="hT")
                            nc.scalar.activation(
                                hT[:], ps2[:], mybir.ActivationFunctionType.Relu,
                                bias=b1t[:, :1],
                            )
                            ps3 = psdn.tile([P, P], f32, space="PSUM", tag="psd", name="psd")
                            nc.tensor.matmul(out=ps3[:], lhsT=hT[:], rhs=w2t[:],
                                             start=True, stop=True)
                            nc.vector.tensor_copy(out_sb[:, ji, :], ps3[:])
                        else:
                            nc.tensor.matmul(out=ps1[:], lhsT=onest[:], rhs=b2t[:],
                                             start=False, stop=True)
                            nc.vector.tensor_copy(out_sb[:, ji, :], ps1[:])
                    g0 = g[0]
                    dst_d = g2own if layer == 1 else y
                    nc.sync.dma_start(
                        dst_d[g0 * P : (g0 + len(g)) * P, :].rearrange("(k p) f -> p k f", p=P),
                        out_sb[:],
                    )
                    col_lo += QLO
                    col_hi += QHI
                    if after_group is not None:
                        after_group(groups.index(g))

            def _one_pass():
                # ---- layer 1 sparse (aggregate x, then W1 -> relu -> W2)
                if stop_after != "skip1":
                    sub = "gather" if stop_after.startswith("g-") else "full"
                    ag_a_group = next(gi for gi, g in enumerate(groups) if SA - 1 in g) \
                        if SA - 1 < BC else len(groups) - 1

                    def _after_group(gi):
                        if gi == ag_a_group:
                            nc.gpsimd.collective_compute(
                                "AllGather", mybir.AluOpType.bypass,
                                replica_groups=[list(range(NCORE))],
                                ins=[g2own[: SA * P, :]], outs=[g2fa[:NA2, :]],
                            )
                    gather_times = 2 if stop_after == "g-2x" else 1
                    for _gt in range(gather_times):
                        sparse_layer(xa, xb, K1_lo, K1_hi, idxlo1, idxhi1, dl1, nm1,
                                     layer=1, sub=sub,
                                     after_group=_after_group if stop_after == "full" else None)

                # ---- allgather g2 (two tables, offset-0 gather sources)
                if stop_after in ("full", "noag"):
                  if stop_after == "full":
                    if NB2 > 0:
                        nc.gpsimd.collective_compute(
                            "AllGather", mybir.AluOpType.bypass,
                            replica_groups=[list(range(NCORE))],
                            ins=[g2own[SA * P :, :]], outs=[g2fb[:NB2, :]],
                        )
                  # ---- layer-2 sparse -> y
                  sparse_layer(g2fa, g2fb, K2_lo, K2_hi, idxlo2, idxhi2, dl2, nm2,
                               layer=2)

            for _rep in range(reps):
                _one_pass()

    nc.compile()
    return nc


# ------------------------------------------------------------------- driver

def _run(x, edge_index, W1, b1, W2, b2, half=HALF_PROD, trace=False, stop_after="full"):
    cfg, in_maps, blk = _preprocess(x, edge_index, W1, b1, W2, b2, half=half)
    nc = build_program(cfg, stop_after=stop_after)
    res = run_bass_kernel_spmd(nc, in_maps, list(range(NCORE)), trace=trace)
    N, BC = cfg["N"], cfg["BC"]
    Y = np.zeros((cfg["NPAD"], P), dtype=np.float32)
    for c in range(NCORE):
        yc = res.results[c]["y"]
        for j in range(BC):
            b = blk[c, j]
            Y[b * P : (b + 1) * P] = yc[j * P : (j + 1) * P]
    return Y[:N], res


def kernel(**inputs):
    x = np.asarray(inputs["x"], dtype=np.float32)
    edge_index = np.asarray(inputs["edge_index"])
    W1 = np.asarray(inputs["W1"], dtype=np.float32)
    b1 = np.asarray(inputs["b1"], dtype=np.float32)
    W2 = np.asarray(inputs["W2"], dtype=np.float32)
    b2 = np.asarray(inputs["b2"], dtype=np.float32)
    Y, _ = _run(x, edge_index, W1, b1, W2, b2)
    return Y
